# revision 4
# baseline (speedup 1.0000x reference)
"""Distributed GQA attention layer (dense_transformer) on 8 TRN2 NeuronCores.

Sharding: 8-way tensor parallel over heads. Core c owns q-heads [4c..4c+4),
kv-head c, and the matching 512 columns/rows of Wq/Wk/Wv/Wo. Each core
computes its heads' attention for both batch rows, the per-core context is
AllGathered (bf16, 4MB/rank), and each core produces a disjoint 512-wide
slice of the output hidden dim via its Wo shard. Host assembles by pure
concatenation.

Layout strategy (per core):
  - hidden^T (bf16, host-pretransposed) streams through SBUF once.
  - QKV projections produce q^T/k^T/v^T [dim, token] directly (weight-
    stationary matmuls, N=512 moving).
  - RoPE applied in [dim, token] layout: partition-swap via a permutation
    matmul on PE, then q*cos + swap*sin on DVE with host-precomputed
    [128, S] tables (sign folded into the sin table, softmax scale folded
    into Wq).
  - Scores are computed transposed: scores^T[s_k, s_q] = k^T.T @ q^T, so
    softmax exp tiles feed PV directly as the moving operand:
    ctx^T[d, s_q] = V[s_k, d].T @ exp[s_k, s_q], with the denominator from
    a parallel ones-vector matmul. Causal masking = skip fully-masked
    chunks + one triangular 128x128 mask on diagonal blocks.
  - o_proj contracts over the gathered [4096, token] context with the Wo
    shard SBUF-resident.
"""
import sys
sys.path.insert(0, "/opt/trn_rl_repo")

import numpy as np
import ml_dtypes

import concourse.bass as bass
import concourse.tile as tile
from concourse import bacc, mybir

BF16 = mybir.dt.bfloat16
F32 = mybir.dt.float32
NPBF16 = ml_dtypes.bfloat16

N_CORES = 8
B, S, HID = 2, 2048, 4096
NH, KVH, D = 32, 8, 128
TOK = B * S                # 4096 tokens, batch-major
QO = NH * D // N_CORES     # 512 q-out dims per core
TT = 512                   # token tile (moving free dim)
NTT = TOK // TT            # 8 token tiles
KC = HID // 128            # 32 contraction chunks


def _build():
    nc = bacc.Bacc("TRN2", target_bir_lowering=False, debug=False,
                   num_devices=N_CORES)
    hid_t = nc.dram_tensor("hid_t", [HID, TOK], BF16, kind="ExternalInput").ap()
    wq_t = nc.dram_tensor("wq_t", [HID, QO], BF16, kind="ExternalInput").ap()
    wk_t = nc.dram_tensor("wk_t", [HID, D], BF16, kind="ExternalInput").ap()
    wv_t = nc.dram_tensor("wv_t", [HID, D], BF16, kind="ExternalInput").ap()
    wo_t = nc.dram_tensor("wo_t", [HID, QO], BF16, kind="ExternalInput").ap()
    cos_t = nc.dram_tensor("cos_t", [D, S], F32, kind="ExternalInput").ap()
    sin_t = nc.dram_tensor("sin_t", [D, S], F32, kind="ExternalInput").ap()
    perm_d = nc.dram_tensor("perm", [128, 128], BF16, kind="ExternalInput").ap()
    ident_d = nc.dram_tensor("ident", [128, 128], BF16, kind="ExternalInput").ap()
    tri_d = nc.dram_tensor("tri", [128, 128], BF16, kind="ExternalInput").ap()
    out = nc.dram_tensor("out", [TOK, QO], F32, kind="ExternalOutput").ap()

    EXP = mybir.ActivationFunctionType.Exp

    with tile.TileContext(nc) as tc:
        with tc.tile_pool(name="const", bufs=1) as cst, \
             tc.tile_pool(name="persist", bufs=1) as per, \
             tc.tile_pool(name="dram", bufs=1, space="DRAM") as dram:
            cos_sb = cst.tile([D, S], F32)
            nc.sync.dma_start(out=cos_sb, in_=cos_t)
            sin_sb = cst.tile([D, S], F32)
            nc.sync.dma_start(out=sin_sb, in_=sin_t)
            perm_sb = cst.tile([128, 128], BF16)
            nc.sync.dma_start(out=perm_sb, in_=perm_d)
            ident_sb = cst.tile([128, 128], BF16)
            nc.sync.dma_start(out=ident_sb, in_=ident_d)
            tri_sb = cst.tile([128, 128], BF16)
            nc.sync.dma_start(out=tri_sb, in_=tri_d)
            ones_sb = cst.tile([128, 1], BF16)
            nc.vector.memset(ones_sb, 1.0)

            q_rope = per.tile([128, 4, TOK], BF16)    # [d, head, token]
            k_rope = per.tile([128, TOK], BF16)       # [d, token]
            v_sb = per.tile([128, KC, 128], BF16)     # [tok%128, tokchunk, d]

            cc_in = dram.tile([QO, TOK], BF16)
            cc_out = dram.tile([N_CORES * QO, TOK], BF16, addr_space="Shared")

            # ---------------- QKV projections + RoPE ----------------
            with tc.tile_pool(name="wqkv", bufs=1) as wp, \
                 tc.tile_pool(name="hin", bufs=2) as hp, \
                 tc.tile_pool(name="qk_ps", bufs=1, space="PSUM") as aps, \
                 tc.tile_pool(name="rope_ps", bufs=1, space="PSUM") as rps, \
                 tc.tile_pool(name="ropesb", bufs=2) as rsb:
                wq_sb = wp.tile([128, KC, QO], BF16)
                nc.sync.dma_start(out=wq_sb,
                                  in_=wq_t.rearrange("(c p) m -> p c m", p=128))
                wk_sb = wp.tile([128, KC, D], BF16)
                nc.sync.dma_start(out=wk_sb,
                                  in_=wk_t.rearrange("(c p) m -> p c m", p=128))
                wv_sb = wp.tile([128, KC, D], BF16)
                nc.sync.dma_start(out=wv_sb,
                                  in_=wv_t.rearrange("(c p) m -> p c m", p=128))

                hid_r = hid_t.rearrange("(c p) t -> p c t", p=128)
                for tt in range(NTT):
                    pos0 = (tt % (S // TT)) * TT
                    h_tile = hp.tile([128, KC, TT], BF16, tag="h")
                    nc.sync.dma_start(out=h_tile,
                                      in_=hid_r[:, :, tt * TT:(tt + 1) * TT])

                    accs = [aps.tile([128, TT], F32, tag=f"acc{i}",
                                     name=f"acc{i}")
                            for i in range(6)]
                    for kc in range(KC):
                        st, sp = kc == 0, kc == KC - 1
                        rhs = h_tile[:, kc, :]
                        for m in range(4):
                            nc.tensor.matmul(
                                accs[m], lhsT=wq_sb[:, kc, m * 128:(m + 1) * 128],
                                rhs=rhs, start=st, stop=sp)
                        nc.tensor.matmul(accs[4], lhsT=wk_sb[:, kc, :], rhs=rhs,
                                         start=st, stop=sp)
                        nc.tensor.matmul(accs[5], lhsT=wv_sb[:, kc, :], rhs=rhs,
                                         start=st, stop=sp)

                    # RoPE for the 4 q chunks + 1 k chunk
                    cs = cos_sb[:, pos0:pos0 + TT]
                    ss = sin_sb[:, pos0:pos0 + TT]
                    for m in range(5):
                        acc = accs[m]
                        xbf = rsb.tile([128, TT], BF16, tag="xbf")
                        nc.scalar.copy(xbf, acc)
                        swp = rps.tile([128, TT], F32, tag="swp")
                        nc.tensor.matmul(swp, lhsT=perm_sb, rhs=xbf,
                                         start=True, stop=True)
                        t2 = rsb.tile([128, TT], F32, tag="t2")
                        nc.vector.tensor_mul(t2, swp, ss)
                        t1 = rsb.tile([128, TT], F32, tag="t1")
                        nc.vector.tensor_mul(t1, acc, cs)
                        if m < 4:
                            dest = q_rope[:, m, tt * TT:(tt + 1) * TT]
                        else:
                            dest = k_rope[:, tt * TT:(tt + 1) * TT]
                        nc.vector.tensor_add(dest, t1, t2)

                    # V: cast + transpose chunks into [token, d] layout
                    vbf = rsb.tile([128, TT], BF16, tag="vbf")
                    nc.scalar.copy(vbf, accs[5])
                    for j in range(4):
                        vtp = rps.tile([128, 128], BF16, tag="vtp")
                        nc.tensor.transpose(vtp, vbf[:, j * 128:(j + 1) * 128],
                                            ident_sb)
                        nc.vector.tensor_copy(v_sb[:, tt * 4 + j, :], vtp)

            # ---------------- attention (scores^T / softmax / PV) ----------------
            with tc.tile_pool(name="at_ps", bufs=1, space="PSUM") as cps, \
                 tc.tile_pool(name="at_sb", bufs=2) as asb:
                for b in range(B):
                    for h in range(4):
                        for t in range(S // TT):
                            tok0 = b * S + t * TT
                            nkc = 4 * t + 4
                            ctx = cps.tile([128, TT], F32, tag="ctx")
                            den = cps.tile([1, TT], F32, tag="den")
                            for kc in range(nkc):
                                r = kc * 128 - t * TT
                                a0 = max(r, 0)
                                sc = cps.tile([128, TT], F32, tag="sc",
                                              bufs=3)
                                nc.tensor.matmul(
                                    sc[:, a0:],
                                    lhsT=k_rope[:, b * S + kc * 128:
                                                b * S + (kc + 1) * 128],
                                    rhs=q_rope[:, h, tok0 + a0:tok0 + TT],
                                    start=True, stop=True)
                                ex = asb.tile([128, TT], BF16, tag="ex", bufs=3)
                                if a0 > 0:
                                    nc.vector.memset(ex[:, 0:a0], 0.0)
                                nc.scalar.activation(ex[:, a0:], sc[:, a0:], EXP)
                                if r >= 0:
                                    nc.vector.tensor_mul(
                                        ex[:, r:r + 128], ex[:, r:r + 128],
                                        tri_sb)
                                st, sp = kc == 0, kc == nkc - 1
                                nc.tensor.matmul(ctx,
                                                 lhsT=v_sb[:, b * 16 + kc, :],
                                                 rhs=ex, start=st, stop=sp)
                                nc.tensor.matmul(den, lhsT=ones_sb, rhs=ex,
                                                 start=st, stop=sp)
                            rden = asb.tile([1, TT], F32, tag="rden")
                            nc.vector.reciprocal(rden, den[0:1, :])
                            bc = asb.tile([128, TT], F32, tag="bc")
                            nc.gpsimd.partition_broadcast(bc, rden)
                            ctxn = asb.tile([128, TT], BF16, tag="ctxn")
                            nc.vector.tensor_mul(ctxn, ctx, bc)
                            nc.sync.dma_start(
                                out=cc_in[h * 128:(h + 1) * 128,
                                          tok0:tok0 + TT],
                                in_=ctxn)

            # ---------------- AllGather context ----------------
            nc.gpsimd.collective_compute(
                "AllGather", mybir.AluOpType.bypass,
                replica_groups=[list(range(N_CORES))],
                ins=[cc_in[:].opt()], outs=[cc_out[:].opt()])

            # ---------------- o_proj ----------------
            with tc.tile_pool(name="wo", bufs=1) as wop, \
                 tc.tile_pool(name="o_ps", bufs=2, space="PSUM") as ops, \
                 tc.tile_pool(name="o_sb", bufs=3) as osb:
                wo_sb = wop.tile([128, KC, QO], BF16)
                nc.sync.dma_start(out=wo_sb,
                                  in_=wo_t.rearrange("(c p) m -> p c m", p=128))
                for mg in range(NTT):
                    om = [ops.tile([128, QO], F32, tag=f"o{m}", name=f"o{m}")
                          for m in range(4)]
                    for kc in range(KC):
                        g = osb.tile([128, TT], BF16, tag="g")
                        nc.sync.dma_start(
                            out=g,
                            in_=cc_out[kc * 128:(kc + 1) * 128,
                                       mg * TT:(mg + 1) * TT])
                        st, sp = kc == 0, kc == KC - 1
                        for m in range(4):
                            nc.tensor.matmul(om[m],
                                             lhsT=g[:, m * 128:(m + 1) * 128],
                                             rhs=wo_sb[:, kc, :],
                                             start=st, stop=sp)
                    for m in range(4):
                        ofin = osb.tile([128, QO], F32, tag="ofin", bufs=2)
                        nc.scalar.copy(ofin, om[m])
                        nc.sync.dma_start(
                            out=out[mg * TT + m * 128:mg * TT + (m + 1) * 128, :],
                            in_=ofin)
    nc.compile()
    return nc


_NC_CACHE = None


def _get_nc():
    global _NC_CACHE
    if _NC_CACHE is None:
        _NC_CACHE = _build()
    return _NC_CACHE


def make_in_maps(hidden_states, position_ids, Wq, Wk, Wv, Wo):
    hs = np.ascontiguousarray(
        np.asarray(hidden_states, dtype=np.float32).reshape(TOK, HID).T
    ).astype(NPBF16)
    pos = np.asarray(position_ids, dtype=np.float32)
    inv = 1.0 / (10000.0 ** (np.arange(0, D, 2, dtype=np.float32) / D))
    fr = pos[:, None] * inv[None, :]                     # [S, 64]
    emb = np.concatenate([fr, fr], axis=-1)              # [S, D]
    cos = np.cos(emb).T.astype(np.float32)               # [D, S]
    sin = np.sin(emb).T.astype(np.float32)
    sin[:64] *= -1.0                                     # fold rotate-half sign
    perm = np.zeros((128, 128), np.float32)
    perm[np.arange(128), (np.arange(128) + 64) % 128] = 1.0
    ident = np.eye(128, dtype=np.float32)
    tri = (np.arange(128)[:, None] <= np.arange(128)[None, :]).astype(np.float32)

    scale = 1.0 / np.sqrt(D)
    Wq = np.asarray(Wq, dtype=np.float32)
    Wk = np.asarray(Wk, dtype=np.float32)
    Wv = np.asarray(Wv, dtype=np.float32)
    Wo = np.asarray(Wo, dtype=np.float32)

    in_maps = []
    for c in range(N_CORES):
        in_maps.append({
            "hid_t": hs,
            "wq_t": np.ascontiguousarray(
                (Wq[c * QO:(c + 1) * QO] * scale).T).astype(NPBF16),
            "wk_t": np.ascontiguousarray(Wk[c * D:(c + 1) * D].T).astype(NPBF16),
            "wv_t": np.ascontiguousarray(Wv[c * D:(c + 1) * D].T).astype(NPBF16),
            "wo_t": np.ascontiguousarray(Wo[c * QO:(c + 1) * QO].T).astype(NPBF16),
            "cos_t": cos,
            "sin_t": sin,
            "perm": perm.astype(NPBF16),
            "ident": ident.astype(NPBF16),
            "tri": tri.astype(NPBF16),
        })
    return in_maps


def assemble(results):
    full = np.empty((TOK, HID), np.float32)
    for c in range(N_CORES):
        full[:, c * QO:(c + 1) * QO] = results[c]["out"]
    return full.reshape(B, S, HID)


def kernel(hidden_states, position_ids, Wq, Wk, Wv, Wo):
    from concourse.bass_utils import run_bass_kernel_spmd
    nc = _get_nc()
    in_maps = make_in_maps(hidden_states, position_ids, Wq, Wk, Wv, Wo)
    res = run_bass_kernel_spmd(nc, in_maps, core_ids=list(range(N_CORES)))
    return assemble(res.results)
